# revision 25
# baseline (speedup 1.0000x reference)
"""Trainium2 Bass kernel for nn_CrossAttention (16x6209x256 cross-attention).

Strategy
--------
Data-parallel over batch: 16 batches -> 8 cores x 2 batches, pure SPMD.

All weight prep happens on the HOST (tiny matmuls, exact fp32):
    mapped_b = b @ Wb + bb                        [256, 64]
    Wf       = 8 * Wa @ mapped_b.T                [256, 256]
    Wout     = mapped_b @ Wc + 1 x bc             [256, 256]
The device computes, per batch:
    scores = a @ Wf  (at a 2^11 PSUM scale) as
        fp16(a)*32 @ fp16(Wf)*64                  (fp16 hi term, 2 matmuls)
      + e4m3(alo*2^11) @ e4m3(Wf)                 } one fp8 DoubleRow pair
      + e4m3(a)        @ e4m3(Wlo*2^11)           } per k-chunk
    attnU  = exp(scores*2^-11 - max)              fp16 (unnormalized)
    sumexp = sum_j attnU                          (DVE reduce, fp16)
    outT   = Wout^T @ attnU^T                     (PE transpose + fp16 matmul)
Host postprocess: out = outT.T / sumexp (+exact bc since sum(attnU)=sumexp).
rel err ~1.7e-3 (validated in simulation against the fp32 reference).

The main loop is software-pipelined: per iteration m the PE stream is
scores(m), final(m-2), transp(m-1) so every cross-engine dependency has a
full iteration of slack; DVE runs sumexp(m-1), attnT copy(m-1), reduce(m);
ACT runs exp(m) then outT(m-2).
"""
import sys

for _p in ("/opt/trn_rl_repo",):
    if _p not in sys.path:
        sys.path.append(_p)

import numpy as np
import ml_dtypes

import concourse.bacc as bacc
import concourse.mybir as mybir
import concourse.tile as tile
from concourse.bass_utils import run_bass_kernel_spmd

F32 = mybir.dt.float32
F16 = mybir.dt.float16
F8 = mybir.dt.float8e4
P = 128

N_CORES = 8
BATCHES_PER_CORE = 2
SEQ = 6209
DF = 256          # feature dim of a / b
HID = 64          # projection dim
DMA_MACRO = 2048  # rows fetched/stored per DMA instruction
CMACRO = 512      # rows per compute macro (4 subtiles of 128)

SC = 2048.0       # 2^11 PSUM score scale
ISC = 1.0 / SC


def _row_plan(n_rows):
    """[(dma_start, dma_len, [(cm_start_within_dma, cm_len), ...]), ...]"""
    plan = []
    pos = 0
    while pos < n_rows:
        d = min(DMA_MACRO, n_rows - pos)
        cms = []
        q = 0
        while q < d:
            c = min(CMACRO, d - q)
            cms.append((q, c))
            q += c
        plan.append((pos, d, cms))
        pos += d
    return plan


def _cmacro_list(seq, batches):
    out = []
    for b in range(batches):
        plan = _row_plan(seq)
        gm = 0
        for ci, (d0, dlen, cms) in enumerate(plan):
            for mi, (mo, R) in enumerate(cms):
                out.append(dict(
                    b=b, d0=d0, dlen=dlen, mo=mo, R=R,
                    chunk=(b, ci),
                    first_in_chunk=(mi == 0), last_in_chunk=(mi == len(cms) - 1),
                    last_in_batch=(ci == len(plan) - 1 and mi == len(cms) - 1),
                    gm=gm,
                ))
                gm += 1
    return out


def build_program(seq=SEQ, batches=BATCHES_PER_CORE, use_ba=False):
    nc = bacc.Bacc("TRN2", target_bir_lowering=False, debug=False)

    a16_d = nc.dram_tensor("a16_d", [batches, 2, P, seq], F16, kind="ExternalInput")
    a8_d = nc.dram_tensor("a8_d", [batches, P, 2, 2, seq], F8, kind="ExternalInput")
    w16_d = nc.dram_tensor("w16_d", [batches, P, 2, DF], F16, kind="ExternalInput")
    w8_d = nc.dram_tensor("w8_d", [batches, P, 2, 2, DF], F8, kind="ExternalInput")
    wo_d = nc.dram_tensor("wo_d", [batches, P, 2, DF], F16, kind="ExternalInput")
    sb_d = nc.dram_tensor("sb_d", [batches, 1, DF], F32, kind="ExternalInput")
    eye_d = nc.dram_tensor("eye_d", [P, P], F16, kind="ExternalInput")
    ones_d = nc.dram_tensor("ones_d", [1, P], F32, kind="ExternalInput")
    out_t = nc.dram_tensor("out_t", [batches, DF, seq], F16, kind="ExternalOutput")
    n_sumcol = 4 * len([c for _, _, cs in _row_plan(seq) for c in cs])
    sum_d = nc.dram_tensor("sum_d", [batches, P, n_sumcol], F16,
                           kind="ExternalOutput")

    Exp = mybir.ActivationFunctionType.Exp
    Copy = mybir.ActivationFunctionType.Copy
    DR = mybir.MatmulPerfMode.DoubleRow

    with tile.TileContext(nc) as tc:
        with (
            tc.tile_pool(name="const", bufs=1) as cpool,
            tc.tile_pool(name="apool", bufs=3) as apool,
            tc.tile_pool(name="mpool", bufs=2) as mpool,
            tc.tile_pool(name="opool", bufs=3) as opool,
            tc.tile_pool(name="pp", bufs=1, space="PSUM") as pp,
        ):
            # ---- constants and per-batch weights (host-prepped) ----
            eye_sb = cpool.tile([P, P], F16)
            nc.sync.dma_start(eye_sb[:], eye_d[:])
            ones_sb = cpool.tile([1, P], F32)
            nc.sync.dma_start(ones_sb[:], ones_d[:])
            w16_sb = cpool.tile([P, batches, 2, DF], F16)
            nc.sync.dma_start(w16_sb[:], w16_d[:].rearrange("b p k j -> p b k j"))
            w8_sb = cpool.tile([P, batches, 2, 2, DF], F8)
            nc.sync.dma_start(w8_sb[:], w8_d[:].rearrange("b p k t j -> p b k t j"))
            wo_sb = cpool.tile([P, batches, 2, DF], F16)
            nc.sync.dma_start(wo_sb[:], wo_d[:].rearrange("b p k f -> p b k f"))
            sbias_sb = cpool.tile([1, batches, DF], F32)
            nc.sync.dma_start(sbias_sb[:], sb_d[:].rearrange("b x j -> x b j"))

            cmac = _cmacro_list(seq, batches)
            M = len(cmac)
            sums = [None] * batches
            ain = {}
            outbuf = {}
            ctx = [None] * M

            def fetch_chunk(cm):
                key = cm["chunk"]
                if key in ain:
                    return
                b, d0, dlen = cm["b"], cm["d0"], cm["dlen"]
                a16_sb = apool.tile([P, 2, DMA_MACRO], F16, tag="a16")
                a8_sb = apool.tile([P, 2, 2, DMA_MACRO], F8, tag="a8")
                # First chunk of the program: land the first cmacro's rows
                # quickly so the pipeline starts early.
                pieces = [(0, CMACRO), (CMACRO, dlen - CMACRO)] \
                    if (key == (0, 0) and dlen > CMACRO) else [(0, dlen)]
                for o, ln in pieces:
                    nc.sync.dma_start(
                        a16_sb[:, :, o:o + ln],
                        a16_d[b][:, :, d0 + o:d0 + o + ln].rearrange(
                            "k p i -> p k i"),
                    )
                    nc.sync.dma_start(
                        a8_sb[:, :, :, o:o + ln],
                        a8_d[b][:, :, :, d0 + o:d0 + o + ln],
                    )
                ain[key] = (a16_sb, a8_sb)

            def subs_of(cm):
                return [(o, min(P, cm["R"] - o)) for o in range(0, cm["R"], P)]

            def stage_scores(m):
                cm = cmac[m]
                b = cm["b"]
                if cm["gm"] == 0 and sums[b] is None:
                    sum_sb = mpool.tile([P, n_sumcol], F16, tag=f"sums{b}", bufs=1)
                    sums[b] = sum_sb
                fetch_chunk(cm)
                nxt = next((c for c in cmac[m + 1:] if c["chunk"] != cm["chunk"]),
                           None)
                if nxt is not None:
                    fetch_chunk(nxt)
                a16_sb, a8_sb = ain[cm["chunk"]]
                subs = subs_of(cm)
                halves = []
                for h, hsubs in ((0, subs[:2]), (1, subs[2:])):
                    if not hsubs:
                        continue
                    sc_ps = pp.tile([P, 2 * DF], F32, tag=f"scores{h}", bufs=2)
                    halves.append(sc_ps)
                    for s, (io, r) in enumerate(hsubs):
                        c0 = s * DF
                        go = cm["mo"] + io
                        for k in range(2):
                            nc.tensor.matmul(
                                sc_ps[:r, c0:c0 + DF],
                                a16_sb[:, k, go:go + r],
                                w16_sb[:, b, k, :],
                                start=(k == 0), stop=False,
                            )
                        for k in range(2):
                            nc.tensor.matmul(
                                sc_ps[:r, c0:c0 + DF],
                                a8_sb[:, k, :, go:go + r],
                                w8_sb[:, b, k, :, :],
                                start=False,
                                stop=(k == 1) and not use_ba,
                                perf_mode=DR,
                            )
                        if use_ba:
                            nc.tensor.matmul(
                                sc_ps[:r, c0:c0 + DF],
                                ones_sb[:, :r],
                                sbias_sb[:, b, :],
                                start=False, stop=True,
                            )
                ctx[m] = dict(scores=halves)

            def stage_stats(m):
                cm = cmac[m]
                subs = subs_of(cm)
                halves = ctx[m]["scores"]
                negmax = mpool.tile([P, 4], F32, tag="negmax")
                ebias = mpool.tile([P, 4], F32, tag="ebias")
                attnU = mpool.tile([P, 4, DF], F16, tag="attnU", bufs=3)
                for h, sc_ps in enumerate(halves):
                    hsubs = subs[2 * h:2 * h + 2]
                    hn = len(hsubs)
                    rmax = max(r for _, r in hsubs)
                    req = all(r == rmax for _, r in hsubs)
                    hb = 2 * h
                    if req:
                        nc.vector.tensor_reduce(
                            negmax[:rmax, hb:hb + hn],
                            sc_ps[:rmax, :hn * DF].rearrange(
                                "p (s j) -> p s j", s=hn),
                            axis=mybir.AxisListType.X,
                            op=mybir.AluOpType.max,
                            negate=True,
                        )
                        nc.gpsimd.tensor_scalar_mul(
                            ebias[:rmax, hb:hb + hn], negmax[:rmax, hb:hb + hn],
                            ISC)
                    else:
                        for s, (io, r) in enumerate(hsubs):
                            nc.vector.tensor_reduce(
                                negmax[:r, hb + s:hb + s + 1],
                                sc_ps[:r, s * DF:(s + 1) * DF],
                                axis=mybir.AxisListType.X,
                                op=mybir.AluOpType.max,
                                negate=True,
                            )
                            nc.gpsimd.tensor_scalar_mul(
                                ebias[:r, hb + s:hb + s + 1],
                                negmax[:r, hb + s:hb + s + 1], ISC)
                    for s, (io, r) in enumerate(hsubs):
                        nc.scalar.activation(
                            attnU[:r, hb + s, :],
                            sc_ps[:r, s * DF:(s + 1) * DF],
                            Exp,
                            bias=ebias[:r, hb + s:hb + s + 1],
                            scale=ISC,
                        )
                ctx[m]["attnU"] = attnU

            def stage_transp(m, mf):
                """Emit transposes of cmacro m interleaved with the final
                matmuls of cmacro mf (one earlier in the pipeline), so the
                transpose weight loads hide under the 512-wide final streams."""
                cm = cmac[m]
                attnU = ctx[m]["attnU"]
                aT_ps = pp.tile([P, 2, CMACRO], F16, tag="attnT", bufs=2)
                tlist = []
                for s, (io, r) in enumerate(subs_of(cm)):
                    rp = r + (r & 1)
                    for jh in range(2):
                        tlist.append((s, io, rp, jh))
                flist = []
                if mf is not None and mf >= 0:
                    cf = cmac[mf]
                    Rf = cf["R"]
                    if cf["first_in_chunk"]:
                        outT_sb = opool.tile([P, 2, DMA_MACRO], F16, tag="outT")
                        outbuf[cf["chunk"]] = outT_sb
                    fin_ps = pp.tile([P, 2, CMACRO], F32, tag="fin", bufs=1)
                    ctx[mf]["fin"] = fin_ps
                    attnTf = ctx[mf]["attnT"]
                    bf = cf["b"]
                    for c in range(2):
                        for k in range(2):
                            flist.append((fin_ps, c, k, Rf, attnTf, bf))
                fi = 0
                for ti, (s, io, rp, jh) in enumerate(tlist):
                    nc.tensor.transpose(
                        aT_ps[:, jh, io:io + rp],
                        attnU[:rp, s, jh * P:(jh + 1) * P],
                        eye_sb[:rp, :rp],
                    )
                    if ti % 2 == 1 and fi < len(flist):
                        fin_ps, c, k, Rf, attnTf, bf = flist[fi]
                        nc.tensor.matmul(
                            fin_ps[:, c, :Rf],
                            wo_sb[:, bf, k, c * P:(c + 1) * P],
                            attnTf[:, k, :Rf],
                            start=(k == 0), stop=(k == 1),
                        )
                        fi += 1
                while fi < len(flist):
                    fin_ps, c, k, Rf, attnTf, bf = flist[fi]
                    nc.tensor.matmul(
                        fin_ps[:, c, :Rf],
                        wo_sb[:, bf, k, c * P:(c + 1) * P],
                        attnTf[:, k, :Rf],
                        start=(k == 0), stop=(k == 1),
                    )
                    fi += 1
                ctx[m]["aT_ps"] = aT_ps

            def stage_attnT_sum(m):
                cm = cmac[m]
                subs = subs_of(cm)
                ns = len(subs)
                gm = cm["gm"]
                attnU = ctx[m]["attnU"]
                sum_sb = sums[cm["b"]]
                rmax = max(r for _, r in subs)
                req = all(r == rmax for _, r in subs)
                with nc.allow_low_precision("fp16 sumexp is plenty (<=2^-11 rel)"):
                    if req:
                        nc.vector.tensor_reduce(
                            sum_sb[:rmax, 4 * gm:4 * gm + ns],
                            attnU[:rmax, :ns, :],
                            axis=mybir.AxisListType.X,
                            op=mybir.AluOpType.add,
                        )
                    else:
                        for s, (io, r) in enumerate(subs):
                            nc.vector.tensor_reduce(
                                sum_sb[:r, 4 * gm + s:4 * gm + s + 1],
                                attnU[:r, s, :],
                                axis=mybir.AxisListType.X,
                                op=mybir.AluOpType.add,
                            )
                attnT = mpool.tile([P, 2, CMACRO], F16, tag="attnTsb")
                nc.vector.tensor_copy(attnT[:, :, :cm["R"]],
                                      ctx[m]["aT_ps"][:, :, :cm["R"]])
                ctx[m]["attnT"] = attnT
                if cm["last_in_batch"]:
                    nc.sync.dma_start(sum_d[cm["b"]], sum_sb[:])

            def stage_final(m):
                cm = cmac[m]
                R = cm["R"]
                b = cm["b"]
                attnT = ctx[m]["attnT"]
                if cm["first_in_chunk"]:
                    outT_sb = opool.tile([P, 2, DMA_MACRO], F16, tag="outT")
                    outbuf[cm["chunk"]] = outT_sb
                fin_ps = pp.tile([P, 2, CMACRO], F32, tag="fin", bufs=1)
                for c in range(2):
                    for k in range(2):
                        nc.tensor.matmul(
                            fin_ps[:, c, :R],
                            wo_sb[:, b, k, c * P:(c + 1) * P],
                            attnT[:, k, :R],
                            start=(k == 0), stop=(k == 1),
                        )
                ctx[m]["fin"] = fin_ps

            def stage_outT(m):
                cm = cmac[m]
                outT_sb = outbuf[cm["chunk"]]
                nc.scalar.activation(
                    outT_sb[:, :, cm["mo"]:cm["mo"] + cm["R"]],
                    ctx[m]["fin"][:, :, :cm["R"]], Copy)
                if cm["last_in_chunk"]:
                    b, d0, dlen = cm["b"], cm["d0"], cm["dlen"]
                    nc.sync.dma_start(
                        out_t[b][:, d0:d0 + dlen].rearrange(
                            "(c p) i -> p c i", p=P),
                        outT_sb[:, :, :dlen],
                    )
                ctx[m] = None

            # ---- software-pipelined emission ----
            # PE: scores(m), final(m-3), transp(m-2)
            # DVE: reduce+ebias(m), sumexp(m-2), attnT copy(m-2)
            # ACT: exp(m), outT(m-3)
            for m in range(M + 3):
                if m < M:
                    stage_scores(m)
                if 2 <= m and m - 2 < M:
                    stage_transp(m - 2, m - 3 if m >= 3 else None)
                elif 3 <= m and m - 3 < M:
                    stage_final(m - 3)
                if m < M:
                    stage_stats(m)
                if 2 <= m and m - 2 < M:
                    stage_attnT_sum(m - 2)
                if 3 <= m and m - 3 < M:
                    stage_outT(m - 3)

    nc.compile()
    return nc


_PROGRAM_CACHE = {}


def _get_program(seq=SEQ, batches=BATCHES_PER_CORE, use_ba=False):
    key = (seq, batches, use_ba)
    if key not in _PROGRAM_CACHE:
        _PROGRAM_CACHE[key] = build_program(seq, batches, use_ba)
    return _PROGRAM_CACHE[key]


def make_in_maps(input_a, input_b, Wa, ba, Wb, bb, Wc, bc,
                 n_cores=N_CORES, batches=BATCHES_PER_CORE):
    input_a = np.asarray(input_a, dtype=np.float32)
    input_b = np.asarray(input_b, dtype=np.float32)
    Wa = np.asarray(Wa, np.float32)
    Wb = np.asarray(Wb, np.float32)
    Wc = np.asarray(Wc, np.float32)
    ba = np.asarray(ba, np.float32)
    bb = np.asarray(bb, np.float32)
    bc = np.asarray(bc, np.float32)

    a_t = np.ascontiguousarray(input_a.transpose(0, 2, 1))      # [B, DF, seq]
    B, _, seq = a_t.shape
    a16 = a_t.astype(np.float16)
    a16s = (a16.astype(np.float32) * 32.0).astype(np.float16)
    alo8 = ((a_t - a16.astype(np.float32)) * SC).astype(ml_dtypes.float8_e4m3)
    a8 = a_t.astype(ml_dtypes.float8_e4m3)
    a8pair = np.empty((B, P, 2, 2, seq), dtype=ml_dtypes.float8_e4m3)
    a8pair[:, :, :, 0, :] = alo8.reshape(B, 2, P, seq).transpose(0, 2, 1, 3)
    a8pair[:, :, :, 1, :] = a8.reshape(B, 2, P, seq).transpose(0, 2, 1, 3)
    a16v = np.ascontiguousarray(a16s.reshape(B, 2, P, seq))

    # host-side weight prep (exact fp32)
    mapped_b = np.einsum("bjf,fh->bjh", input_b, Wb) + bb       # [B, 256, 64]
    wf = 8.0 * np.einsum("fh,bjh->bfj", Wa, mapped_b)           # [B, 256(f), 256(j)]
    w16 = wf.astype(np.float16).astype(np.float32)
    w16s = (w16 * 64.0).astype(np.float16)                      # fp16(Wf)*64
    wlo8 = ((wf - w16) * SC).astype(ml_dtypes.float8_e4m3)
    w8 = wf.astype(ml_dtypes.float8_e4m3)
    # [B, 128(f%128), 2(fchunk), 2(w8|wlo8), 256(j)]
    w8pair = np.empty((B, P, 2, 2, DF), dtype=ml_dtypes.float8_e4m3)
    w8pair[:, :, :, 0, :] = w8.reshape(B, 2, P, DF).transpose(0, 2, 1, 3)
    w8pair[:, :, :, 1, :] = wlo8.reshape(B, 2, P, DF).transpose(0, 2, 1, 3)
    w16v = np.ascontiguousarray(
        w16s.reshape(B, 2, P, DF).transpose(0, 2, 1, 3))        # [B, 128, 2, 256]

    wout = np.einsum("bjh,hf->bjf", mapped_b, Wc) + bc          # [B, 256(j), 256(f)]
    wo16 = np.ascontiguousarray(
        wout.astype(np.float16).reshape(B, 2, P, DF).transpose(0, 2, 1, 3))
    sbias = (8.0 * SC) * np.einsum("h,bjh->bj", ba, mapped_b)   # [B, 256(j)]
    sbias = np.ascontiguousarray(sbias.reshape(B, 1, DF).astype(np.float32))

    shared = {
        "eye_d": np.eye(P, dtype=np.float16),
        "ones_d": np.ones((1, P), dtype=np.float32),
    }
    in_maps = []
    for c in range(n_cores):
        lo, hi = c * batches, (c + 1) * batches
        in_maps.append({
            "a16_d": np.ascontiguousarray(a16v[lo:hi]),
            "a8_d": np.ascontiguousarray(a8pair[lo:hi]),
            "w16_d": np.ascontiguousarray(w16v[lo:hi]),
            "w8_d": np.ascontiguousarray(w8pair[lo:hi]),
            "wo_d": np.ascontiguousarray(wo16[lo:hi]),
            "sb_d": np.ascontiguousarray(sbias[lo:hi]),
            **shared,
        })
    return in_maps


def _postprocess(results, seq=SEQ, batches=BATCHES_PER_CORE):
    """Concatenate per-core outputs, transpose, and normalize by sumexp."""
    outs = np.concatenate(
        [np.asarray(r["out_t"], dtype=np.float32) for r in results], axis=0)
    sums = np.concatenate(
        [np.asarray(r["sum_d"], dtype=np.float32) for r in results], axis=0)
    B = outs.shape[0]
    # sums[b, p, 4*gm + s] -> row i = 512*gm + 128*s + p
    n_cm = sums.shape[2] // 4
    grid = sums.reshape(B, P, n_cm, 4).transpose(0, 2, 3, 1).reshape(B, -1)
    se = grid[:, :seq]                                  # [B, seq]
    out = outs.transpose(0, 2, 1) / se[:, :, None]
    return np.ascontiguousarray(out.astype(np.float32))


def kernel(input_a, input_b, Wa, ba, Wb, bb, Wc, bc):
    use_ba = bool(np.any(np.asarray(ba)))
    nc = _get_program(use_ba=use_ba)
    in_maps = make_in_maps(input_a, input_b, Wa, ba, Wb, bb, Wc, bc)
    res = run_bass_kernel_spmd(nc, in_maps, core_ids=list(range(N_CORES)))
    return _postprocess(res.results)


# revision 29
# speedup vs baseline: 1.1810x; 1.1810x over previous
"""Trainium2 Bass kernel for nn_CrossAttention (16x6209x256 cross-attention).

Strategy
--------
Data-parallel over batch: 16 batches -> 8 cores x 2 batches, pure SPMD.

All weight prep happens on the HOST (tiny matmuls, exact fp32):
    mapped_b = b @ Wb + bb                        [256, 64]
    Wf       = 8 * Wa @ mapped_b.T                [256, 256]
    Wout     = mapped_b @ Wc + 1 x bc             [256, 256]
The device computes, per batch:
    scores = a @ Wf  (at a 2^11 PSUM scale) as
        fp16(a)*32 @ fp16(Wf)*64                  (fp16 hi term, 2 matmuls)
      + e4m3(alo*2^11) @ e4m3(Wf)                 } one fp8 DoubleRow pair
      + e4m3(a)        @ e4m3(Wlo*2^11)           } per k-chunk
    attnU  = exp(scores*2^-11 - max)              fp16 (unnormalized)
    sumexp = sum_j attnU                          (DVE reduce, fp16)
    outT   = Wout^T @ attnU^T                     (PE transpose + fp16 matmul)
Host postprocess: out = outT.T / sumexp (+exact bc since sum(attnU)=sumexp).
rel err ~1.7e-3 (validated in simulation against the fp32 reference).

The main loop is software-pipelined: per iteration m the PE stream is
scores(m), final(m-2), transp(m-1) so every cross-engine dependency has a
full iteration of slack; DVE runs sumexp(m-1), attnT copy(m-1), reduce(m);
ACT runs exp(m) then outT(m-2).
"""
import sys

for _p in ("/opt/trn_rl_repo",):
    if _p not in sys.path:
        sys.path.append(_p)

import numpy as np
import ml_dtypes

import concourse.bacc as bacc
import concourse.mybir as mybir
import concourse.tile as tile
from concourse.bass_utils import run_bass_kernel_spmd

F32 = mybir.dt.float32
F16 = mybir.dt.float16
F8 = mybir.dt.float8e4
P = 128

N_CORES = 8
BATCHES_PER_CORE = 2
SEQ = 6209
DF = 256          # feature dim of a / b
HID = 64          # projection dim
DMA_MACRO = 2048  # rows fetched/stored per DMA instruction
CMACRO = 512      # rows per compute macro (4 subtiles of 128)

SC = 2048.0       # 2^11 PSUM score scale
ISC = 1.0 / SC


def _row_plan(n_rows):
    """[(dma_start, dma_len, [(cm_start_within_dma, cm_len), ...]), ...]"""
    plan = []
    pos = 0
    while pos < n_rows:
        d = min(DMA_MACRO, n_rows - pos)
        cms = []
        q = 0
        while q < d:
            c = min(CMACRO, d - q)
            cms.append((q, c))
            q += c
        plan.append((pos, d, cms))
        pos += d
    return plan


def _cmacro_list(seq, batches):
    out = []
    for b in range(batches):
        plan = _row_plan(seq)
        gm = 0
        for ci, (d0, dlen, cms) in enumerate(plan):
            for mi, (mo, R) in enumerate(cms):
                out.append(dict(
                    b=b, d0=d0, dlen=dlen, mo=mo, R=R,
                    chunk=(b, ci),
                    first_in_chunk=(mi == 0), last_in_chunk=(mi == len(cms) - 1),
                    last_in_batch=(ci == len(plan) - 1 and mi == len(cms) - 1),
                    gm=gm,
                ))
                gm += 1
    return out


def build_program(seq=SEQ, batches=BATCHES_PER_CORE, use_ba=False):
    nc = bacc.Bacc("TRN2", target_bir_lowering=False, debug=False)

    a16_d = nc.dram_tensor("a16_d", [batches, 2, P, seq], F16, kind="ExternalInput")
    a8_d = nc.dram_tensor("a8_d", [batches, P, 2, 2, seq], F8, kind="ExternalInput")
    w16_d = nc.dram_tensor("w16_d", [batches, P, 2, DF], F16, kind="ExternalInput")
    w8_d = nc.dram_tensor("w8_d", [batches, P, 2, 2, DF], F8, kind="ExternalInput")
    wo_d = nc.dram_tensor("wo_d", [batches, P, 2, DF], F16, kind="ExternalInput")
    sb_d = nc.dram_tensor("sb_d", [batches, 1, DF], F32, kind="ExternalInput")
    eye_d = nc.dram_tensor("eye_d", [P, P], F16, kind="ExternalInput")
    ones_d = nc.dram_tensor("ones_d", [1, P], F32, kind="ExternalInput")
    out_t = nc.dram_tensor("out_t", [batches, DF, seq], F16, kind="ExternalOutput")
    n_sumcol = 4 * len([c for _, _, cs in _row_plan(seq) for c in cs])
    sum_d = nc.dram_tensor("sum_d", [batches, P, n_sumcol], F16,
                           kind="ExternalOutput")

    Exp = mybir.ActivationFunctionType.Exp
    Copy = mybir.ActivationFunctionType.Copy
    DR = mybir.MatmulPerfMode.DoubleRow

    with tile.TileContext(nc) as tc:
        with (
            tc.tile_pool(name="const", bufs=1) as cpool,
            tc.tile_pool(name="apool", bufs=3) as apool,
            tc.tile_pool(name="mpool", bufs=2) as mpool,
            tc.tile_pool(name="opool", bufs=3) as opool,
            tc.tile_pool(name="pp", bufs=1, space="PSUM") as pp,
        ):
            # ---- constants and per-batch weights (host-prepped) ----
            eye_sb = cpool.tile([P, P], F16)
            nc.sync.dma_start(eye_sb[:], eye_d[:])
            ones_sb = cpool.tile([1, P], F32)
            nc.sync.dma_start(ones_sb[:], ones_d[:])
            w16_sb = cpool.tile([P, batches, 2, DF], F16)
            nc.sync.dma_start(w16_sb[:], w16_d[:].rearrange("b p k j -> p b k j"))
            w8_sb = cpool.tile([P, batches, 2, 2, DF], F8)
            nc.sync.dma_start(w8_sb[:], w8_d[:].rearrange("b p k t j -> p b k t j"))
            wo_sb = cpool.tile([P, batches, 2, DF], F16)
            nc.sync.dma_start(wo_sb[:], wo_d[:].rearrange("b p k f -> p b k f"))
            sbias_sb = cpool.tile([1, batches, DF], F32)
            nc.sync.dma_start(sbias_sb[:], sb_d[:].rearrange("b x j -> x b j"))

            cmac = _cmacro_list(seq, batches)
            M = len(cmac)
            sums = [None] * batches
            ain = {}
            outbuf = {}
            ctx = [None] * M

            def fetch_chunk(cm):
                key = cm["chunk"]
                if key in ain:
                    return
                b, d0, dlen = cm["b"], cm["d0"], cm["dlen"]
                a16_sb = apool.tile([P, 2, DMA_MACRO], F16, tag="a16")
                a8_sb = apool.tile([P, 2, 2, DMA_MACRO], F8, tag="a8")
                # First chunk of the program: land the first cmacro's rows
                # quickly so the pipeline starts early.
                pieces = [(0, CMACRO), (CMACRO, dlen - CMACRO)] \
                    if (key == (0, 0) and dlen > CMACRO) else [(0, dlen)]
                for o, ln in pieces:
                    nc.sync.dma_start(
                        a16_sb[:, :, o:o + ln],
                        a16_d[b][:, :, d0 + o:d0 + o + ln].rearrange(
                            "k p i -> p k i"),
                    )
                    nc.sync.dma_start(
                        a8_sb[:, :, :, o:o + ln],
                        a8_d[b][:, :, :, d0 + o:d0 + o + ln],
                    )
                ain[key] = (a16_sb, a8_sb)

            def subs_of(cm):
                return [(o, min(P, cm["R"] - o)) for o in range(0, cm["R"], P)]

            def stage_scores(m):
                cm = cmac[m]
                b = cm["b"]
                if cm["gm"] == 0 and sums[b] is None:
                    sum_sb = mpool.tile([P, n_sumcol], F16, tag=f"sums{b}", bufs=1)
                    sums[b] = sum_sb
                fetch_chunk(cm)
                nxt = next((c for c in cmac[m + 1:] if c["chunk"] != cm["chunk"]),
                           None)
                if nxt is not None:
                    fetch_chunk(nxt)
                a16_sb, a8_sb = ain[cm["chunk"]]
                subs = subs_of(cm)
                halves = []
                for h, hsubs in ((0, subs[:2]), (1, subs[2:])):
                    if not hsubs:
                        continue
                    sc_ps = pp.tile([P, 2 * DF], F32, tag=f"scores{h}", bufs=2)
                    halves.append(sc_ps)
                    for s, (io, r) in enumerate(hsubs):
                        c0 = s * DF
                        go = cm["mo"] + io
                        for k in range(2):
                            nc.tensor.matmul(
                                sc_ps[:r, c0:c0 + DF],
                                a16_sb[:, k, go:go + r],
                                w16_sb[:, b, k, :],
                                start=(k == 0), stop=False,
                            )
                        for k in range(2):
                            nc.tensor.matmul(
                                sc_ps[:r, c0:c0 + DF],
                                a8_sb[:, k, :, go:go + r],
                                w8_sb[:, b, k, :, :],
                                start=False,
                                stop=(k == 1) and not use_ba,
                                perf_mode=DR,
                            )
                        if use_ba:
                            nc.tensor.matmul(
                                sc_ps[:r, c0:c0 + DF],
                                ones_sb[:, :r],
                                sbias_sb[:, b, :],
                                start=False, stop=True,
                            )
                ctx[m] = dict(scores=halves)

            def stage_stats(m):
                cm = cmac[m]
                subs = subs_of(cm)
                halves = ctx[m]["scores"]
                negmax = mpool.tile([P, 4], F32, tag="negmax")
                ebias = mpool.tile([P, 4], F32, tag="ebias")
                attnU = mpool.tile([P, 4, DF], F16, tag="attnU", bufs=3)
                for h, sc_ps in enumerate(halves):
                    hsubs = subs[2 * h:2 * h + 2]
                    hn = len(hsubs)
                    rmax = max(r for _, r in hsubs)
                    req = all(r == rmax for _, r in hsubs)
                    hb = 2 * h
                    if req:
                        nc.vector.tensor_reduce(
                            negmax[:rmax, hb:hb + hn],
                            sc_ps[:rmax, :hn * DF].rearrange(
                                "p (s j) -> p s j", s=hn),
                            axis=mybir.AxisListType.X,
                            op=mybir.AluOpType.max,
                            negate=True,
                        )
                        nc.vector.tensor_scalar_mul(
                            ebias[:rmax, hb:hb + hn], negmax[:rmax, hb:hb + hn],
                            ISC)
                    else:
                        for s, (io, r) in enumerate(hsubs):
                            nc.vector.tensor_reduce(
                                negmax[:r, hb + s:hb + s + 1],
                                sc_ps[:r, s * DF:(s + 1) * DF],
                                axis=mybir.AxisListType.X,
                                op=mybir.AluOpType.max,
                                negate=True,
                            )
                            nc.vector.tensor_scalar_mul(
                                ebias[:r, hb + s:hb + s + 1],
                                negmax[:r, hb + s:hb + s + 1], ISC)
                    for s, (io, r) in enumerate(hsubs):
                        nc.scalar.activation(
                            attnU[:r, hb + s, :],
                            sc_ps[:r, s * DF:(s + 1) * DF],
                            Exp,
                            bias=ebias[:r, hb + s:hb + s + 1],
                            scale=ISC,
                        )
                ctx[m]["attnU"] = attnU

            def stage_transp(m):
                cm = cmac[m]
                attnU = ctx[m]["attnU"]
                aT_ps = pp.tile([P, 2, CMACRO], F16, tag="attnT", bufs=2)
                for s, (io, r) in enumerate(subs_of(cm)):
                    rp = r + (r & 1)
                    for jh in range(2):
                        nc.tensor.transpose(
                            aT_ps[:, jh, io:io + rp],
                            attnU[:rp, s, jh * P:(jh + 1) * P],
                            eye_sb[:rp, :rp],
                        )
                ctx[m]["aT_ps"] = aT_ps

            def stage_attnT_sum(m):
                cm = cmac[m]
                subs = subs_of(cm)
                ns = len(subs)
                gm = cm["gm"]
                attnU = ctx[m]["attnU"]
                sum_sb = sums[cm["b"]]
                rmax = max(r for _, r in subs)
                req = all(r == rmax for _, r in subs)
                with nc.allow_low_precision("fp16 sumexp is plenty (<=2^-11 rel)"):
                    if req:
                        nc.vector.tensor_reduce(
                            sum_sb[:rmax, 4 * gm:4 * gm + ns],
                            attnU[:rmax, :ns, :],
                            axis=mybir.AxisListType.X,
                            op=mybir.AluOpType.add,
                        )
                    else:
                        for s, (io, r) in enumerate(subs):
                            nc.vector.tensor_reduce(
                                sum_sb[:r, 4 * gm + s:4 * gm + s + 1],
                                attnU[:r, s, :],
                                axis=mybir.AxisListType.X,
                                op=mybir.AluOpType.add,
                            )
                attnT = mpool.tile([P, 2, CMACRO], F16, tag="attnTsb")
                nc.vector.tensor_copy(attnT[:, :, :cm["R"]],
                                      ctx[m]["aT_ps"][:, :, :cm["R"]])
                ctx[m]["attnT"] = attnT
                if cm["last_in_batch"]:
                    nc.sync.dma_start(sum_d[cm["b"]], sum_sb[:])

            def stage_final(m):
                cm = cmac[m]
                R = cm["R"]
                b = cm["b"]
                attnT = ctx[m]["attnT"]
                if cm["first_in_chunk"]:
                    outT_sb = opool.tile([P, 2, DMA_MACRO], F16, tag="outT")
                    outbuf[cm["chunk"]] = outT_sb
                fin_ps = pp.tile([P, 2, CMACRO], F32, tag="fin", bufs=1)
                for c in range(2):
                    for k in range(2):
                        nc.tensor.matmul(
                            fin_ps[:, c, :R],
                            wo_sb[:, b, k, c * P:(c + 1) * P],
                            attnT[:, k, :R],
                            start=(k == 0), stop=(k == 1),
                        )
                ctx[m]["fin"] = fin_ps

            def stage_outT(m):
                cm = cmac[m]
                outT_sb = outbuf[cm["chunk"]]
                nc.scalar.activation(
                    outT_sb[:, :, cm["mo"]:cm["mo"] + cm["R"]],
                    ctx[m]["fin"][:, :, :cm["R"]], Copy)
                if cm["last_in_chunk"]:
                    b, d0, dlen = cm["b"], cm["d0"], cm["dlen"]
                    nc.sync.dma_start(
                        out_t[b][:, d0:d0 + dlen].rearrange(
                            "(c p) i -> p c i", p=P),
                        outT_sb[:, :, :dlen],
                    )
                ctx[m] = None

            # ---- software-pipelined emission ----
            # PE: scores(m), final(m-3), transp(m-2)
            # DVE: reduce+ebias(m), sumexp(m-2), attnT copy(m-2)
            # ACT: exp(m), outT(m-3)
            for m in range(M + 3):
                if m < M:
                    stage_scores(m)
                if 3 <= m and m - 3 < M:
                    stage_final(m - 3)
                if 2 <= m and m - 2 < M:
                    stage_transp(m - 2)
                if m < M:
                    stage_stats(m)
                if 2 <= m and m - 2 < M:
                    stage_attnT_sum(m - 2)
                if 3 <= m and m - 3 < M:
                    stage_outT(m - 3)

    nc.compile()
    return nc


_PROGRAM_CACHE = {}


def _get_program(seq=SEQ, batches=BATCHES_PER_CORE, use_ba=False):
    key = (seq, batches, use_ba)
    if key not in _PROGRAM_CACHE:
        _PROGRAM_CACHE[key] = build_program(seq, batches, use_ba)
    return _PROGRAM_CACHE[key]


def make_in_maps(input_a, input_b, Wa, ba, Wb, bb, Wc, bc,
                 n_cores=N_CORES, batches=BATCHES_PER_CORE):
    input_a = np.asarray(input_a, dtype=np.float32)
    input_b = np.asarray(input_b, dtype=np.float32)
    Wa = np.asarray(Wa, np.float32)
    Wb = np.asarray(Wb, np.float32)
    Wc = np.asarray(Wc, np.float32)
    ba = np.asarray(ba, np.float32)
    bb = np.asarray(bb, np.float32)
    bc = np.asarray(bc, np.float32)

    a_t = np.ascontiguousarray(input_a.transpose(0, 2, 1))      # [B, DF, seq]
    B, _, seq = a_t.shape
    a16 = a_t.astype(np.float16)
    a16s = (a16.astype(np.float32) * 32.0).astype(np.float16)
    alo8 = ((a_t - a16.astype(np.float32)) * SC).astype(ml_dtypes.float8_e4m3)
    a8 = a_t.astype(ml_dtypes.float8_e4m3)
    a8pair = np.empty((B, P, 2, 2, seq), dtype=ml_dtypes.float8_e4m3)
    a8pair[:, :, :, 0, :] = alo8.reshape(B, 2, P, seq).transpose(0, 2, 1, 3)
    a8pair[:, :, :, 1, :] = a8.reshape(B, 2, P, seq).transpose(0, 2, 1, 3)
    a16v = np.ascontiguousarray(a16s.reshape(B, 2, P, seq))

    # host-side weight prep (exact fp32)
    mapped_b = np.einsum("bjf,fh->bjh", input_b, Wb) + bb       # [B, 256, 64]
    wf = 8.0 * np.einsum("fh,bjh->bfj", Wa, mapped_b)           # [B, 256(f), 256(j)]
    w16 = wf.astype(np.float16).astype(np.float32)
    w16s = (w16 * 64.0).astype(np.float16)                      # fp16(Wf)*64
    wlo8 = ((wf - w16) * SC).astype(ml_dtypes.float8_e4m3)
    w8 = wf.astype(ml_dtypes.float8_e4m3)
    # [B, 128(f%128), 2(fchunk), 2(w8|wlo8), 256(j)]
    w8pair = np.empty((B, P, 2, 2, DF), dtype=ml_dtypes.float8_e4m3)
    w8pair[:, :, :, 0, :] = w8.reshape(B, 2, P, DF).transpose(0, 2, 1, 3)
    w8pair[:, :, :, 1, :] = wlo8.reshape(B, 2, P, DF).transpose(0, 2, 1, 3)
    w16v = np.ascontiguousarray(
        w16s.reshape(B, 2, P, DF).transpose(0, 2, 1, 3))        # [B, 128, 2, 256]

    wout = np.einsum("bjh,hf->bjf", mapped_b, Wc) + bc          # [B, 256(j), 256(f)]
    wo16 = np.ascontiguousarray(
        wout.astype(np.float16).reshape(B, 2, P, DF).transpose(0, 2, 1, 3))
    sbias = (8.0 * SC) * np.einsum("h,bjh->bj", ba, mapped_b)   # [B, 256(j)]
    sbias = np.ascontiguousarray(sbias.reshape(B, 1, DF).astype(np.float32))

    shared = {
        "eye_d": np.eye(P, dtype=np.float16),
        "ones_d": np.ones((1, P), dtype=np.float32),
    }
    in_maps = []
    for c in range(n_cores):
        lo, hi = c * batches, (c + 1) * batches
        in_maps.append({
            "a16_d": np.ascontiguousarray(a16v[lo:hi]),
            "a8_d": np.ascontiguousarray(a8pair[lo:hi]),
            "w16_d": np.ascontiguousarray(w16v[lo:hi]),
            "w8_d": np.ascontiguousarray(w8pair[lo:hi]),
            "wo_d": np.ascontiguousarray(wo16[lo:hi]),
            "sb_d": np.ascontiguousarray(sbias[lo:hi]),
            **shared,
        })
    return in_maps


def _postprocess(results, seq=SEQ, batches=BATCHES_PER_CORE):
    """Concatenate per-core outputs, transpose, and normalize by sumexp."""
    outs = np.concatenate(
        [np.asarray(r["out_t"], dtype=np.float32) for r in results], axis=0)
    sums = np.concatenate(
        [np.asarray(r["sum_d"], dtype=np.float32) for r in results], axis=0)
    B = outs.shape[0]
    # sums[b, p, 4*gm + s] -> row i = 512*gm + 128*s + p
    n_cm = sums.shape[2] // 4
    grid = sums.reshape(B, P, n_cm, 4).transpose(0, 2, 3, 1).reshape(B, -1)
    se = grid[:, :seq]                                  # [B, seq]
    out = outs.transpose(0, 2, 1) / se[:, :, None]
    return np.ascontiguousarray(out.astype(np.float32))


def kernel(input_a, input_b, Wa, ba, Wb, bb, Wc, bc):
    use_ba = bool(np.any(np.asarray(ba)))
    nc = _get_program(use_ba=use_ba)
    in_maps = make_in_maps(input_a, input_b, Wa, ba, Wb, bb, Wc, bc)
    res = run_bass_kernel_spmd(nc, in_maps, core_ids=list(range(N_CORES)))
    return _postprocess(res.results)
